# revision 23
# baseline (speedup 1.0000x reference)
"""Trainium2 Bass kernel for batched ODE dynamics:
out = tanh(y @ W1a) @ W1b + tanh(tril(y x y) @ W2a) @ W2b (+ biases)

Data parallel over B=131072 across 8 cores (BC=16384/core), 32 chunks of 512.

Strategy (v2): fp8e4m3 DoubleRow matmuls (0.5 cyc per output column in the
cost model vs 1.0 for fp32r) with split-precision operands so accuracy stays
~2e-3 despite fp8:
  - y is host-split into ya+yb (two fp8 words ~ bf16+ precision). Gathers
    (R|R)@(ya|yb) reconstruct exact-ish y rows in f32 PSUM: 9 DR insts/chunk.
  - quad tiles are built by DVE/Pool (mult, then qa=fp8(qf), qb=fp8(qf-qa));
    W2a is host-split into wa+wb (scaled x16 to dodge fp8 subnormals; undone
    by tanh's input scale). mm2a computes qa@wa+qb@wa per k-tile in one DR
    inst (moving = the natural [128,2,512] (qa|qb) tile) plus qa@wb with
    k-tiles paired: 8 insts per M-tile, 48/chunk. Dropped qb@wb ~ eps^2.
  - mm2b is flipped: stationary = tanh-out tiles (bf16), moving = W2b (bf16,
    [126,32]) so each matmul costs only 32 cycles: 24 insts/chunk.
  - biases (all exactly representable/zero here) fold in as extra K rows; the
    output bias row rides on a tanh-saturated ones row.
PE: (9+48)*256 + 24*32 = 15360 cyc/chunk vs 23040 for the fp32r baseline.
"""

import numpy as np

B = 131072
D = 32
H1 = 50
Q = 528
H2 = 700
N_CORES = 8
BC = B // N_CORES        # 16384 rows per core
CHUNK = 512
NCH = BC // CHUNK        # 32 chunks
NMT = 6                  # M-tiles of mm2a (750 h-cols -> 6x125, +1 ones col)
MT = 128                 # cols per M-tile (125 real + ones col + pad)
KT = [128, 128, 128, 128, 49]  # k-tiles: 528 quad + 32 y + 1 ones = 561
SCALE_W = 16.0           # host scale on W2a/W1a, undone by tanh input scale
ONES_COL_RAW = 192.0     # psum value driving the tanh-ones row (tanh(12)~=1)

_CACHE = {}


def _build_nc(opts=None):
    opts = opts or {}
    import concourse.bass as bass  # noqa: F401
    import concourse.mybir as mybir
    import concourse.tile as tile
    from concourse import bacc

    f32 = mybir.dt.float32
    bf16 = mybir.dt.bfloat16
    f8 = mybir.dt.float8e4
    DR = mybir.MatmulPerfMode.DoubleRow
    Tanh = mybir.ActivationFunctionType.Tanh
    MULT = mybir.AluOpType.mult
    SUB = mybir.AluOpType.subtract

    nc = bacc.Bacc("TRN2", target_bir_lowering=False, debug=False)

    yT8 = nc.dram_tensor("yT8", [33, 2, BC], f8, kind="ExternalInput")
    yT16 = nc.dram_tensor("yT16", [33, BC], bf16, kind="ExternalInput")
    W2A8 = nc.dram_tensor("W2A8", [128, NMT, 6, 2, MT], f8, kind="ExternalInput")
    W2A4T = nc.dram_tensor("W2A4T", [64, NMT, MT], bf16, kind="ExternalInput")
    W2B16 = nc.dram_tensor("W2B16", [128, NMT, D], bf16, kind="ExternalInput")
    RC8 = nc.dram_tensor("RC8", [32, 9, 2, 128], f8, kind="ExternalInput")
    OUT = nc.dram_tensor("out", [128, NCH, 4, D], f32, kind="ExternalOutput")


    with tile.TileContext(nc) as tc:
        with (
            tc.tile_pool(name="const", bufs=1) as cpool,
            tc.tile_pool(name="io", bufs=opts.get("io_bufs", 4)) as io,
            tc.tile_pool(name="q8", bufs=opts.get("q8_bufs", 4)) as q8p,
            tc.tile_pool(name="q4", bufs=opts.get("q4_bufs", 4)) as q4p,
            tc.tile_pool(name="qf", bufs=opts.get("qf_bufs", 4)) as qfp,
            tc.tile_pool(name="h2", bufs=opts.get("h2_bufs", 14)) as h2p,
            tc.tile_pool(name="ost", bufs=2) as osp,
            tc.tile_pool(name="gpa", bufs=opts.get("gpa_bufs", 3), space="PSUM") as gpa,
            tc.tile_pool(name="gpb", bufs=opts.get("gpb_bufs", 2), space="PSUM") as gpb,
            tc.tile_pool(name="hps", bufs=opts.get("hps_bufs", 2), space="PSUM") as hps,
            tc.tile_pool(name="ops", bufs=1, space="PSUM") as opsp,
        ):
            w2a_sb = cpool.tile([128, NMT, 6, 2, MT], f8, tag="w2a")
            nc.sync.dma_start(w2a_sb[:], W2A8[:, :, :, :, :])
            w2a4_sb = cpool.tile([64, NMT, MT], bf16, tag="w2a4")
            nc.sync.dma_start(w2a4_sb[:], W2A4T[:, :, :])
            w2b_sb = cpool.tile([128, NMT, D], bf16, tag="w2b")
            nc.sync.dma_start(w2b_sb[:], W2B16[:, :, :])
            rc8_sb = cpool.tile([32, 9, 2, 128], f8, tag="rc8")
            nc.sync.dma_start(rc8_sb[:], RC8[:, :, :, :])

            def load(ch):
                sl = slice(ch * CHUNK, (ch + 1) * CHUNK)
                yt = io.tile([33, 2, CHUNK], f8, tag="yt")
                nc.sync.dma_start(yt[:, :, :], yT8[:, :, sl])
                Q8 = q8p.tile([128, 4, 2, CHUNK], f8, tag="q8")
                Q4 = q4p.tile([49, CHUNK], bf16, tag="q4")
                # y passthrough rows (net1 input) + ones row, via DMA (bf16)
                nc.sync.dma_start(Q4[16:49, :], yT16[:, sl])
                return ch, yt, Q8, Q4

            def quad_tile(state, t):
                # emit the PE gathers + elementwise quad-split for k-tile t
                ch, yt, Q8, Q4 = state
                if t < 4:
                    a_ps = gpa.tile([128, CHUNK], f32, tag="aps")
                    b_ps = gpb.tile([128, CHUNK], f32, tag="bps")
                    nc.tensor.matmul(
                        a_ps[:, :], rc8_sb[:, 2 * t, :, :], yt[0:32, :, :],
                        start=True, stop=True, perf_mode=DR,
                    )
                    nc.tensor.matmul(
                        b_ps[:, :], rc8_sb[:, 2 * t + 1, :, :], yt[0:32, :, :],
                        start=True, stop=True, perf_mode=DR,
                    )
                    qf = qfp.tile([128, CHUNK], f32, tag="qf")
                    b_sb = qfp.tile([128, CHUNK], f32, tag="bsb")
                    # GPSIMD cannot access PSUM: b-copy on Act/DVE only
                    if t in opts.get("act_bcopy", (0, 2)):
                        nc.scalar.copy(b_sb[:, :], b_ps[:, :])
                    else:
                        nc.vector.tensor_copy(b_sb[:, :], b_ps[:, :])
                    nc.vector.tensor_tensor(
                        qf[:, :], a_ps[:, :], b_sb[:, :], MULT
                    )
                    if t % 2 == 0:
                        nc.vector.tensor_copy(Q8[:, t, 0, :], qf[:, :])
                    else:
                        nc.gpsimd.tensor_copy(Q8[:, t, 0, :], qf[:, :])
                    if t in opts.get("dve_subs", (0,)):
                        nc.vector.tensor_tensor(
                            Q8[:, t, 1, :], qf[:, :], Q8[:, t, 0, :], SUB
                        )
                    else:
                        nc.gpsimd.tensor_tensor(
                            Q8[:, t, 1, :], qf[:, :], Q8[:, t, 0, :], SUB
                        )
                else:
                    # tile 4: 16 quad rows (r=31, c<16) in bf16 — no split
                    a_ps = gpa.tile([128, CHUNK], f32, tag="aps")
                    nc.tensor.matmul(
                        a_ps[:, :], rc8_sb[:, 8, :, :], yt[0:32, :, :],
                        start=True, stop=True, perf_mode=DR,
                    )
                    nc.vector.tensor_tensor(
                        Q4[0:16, :], a_ps[0:16, :], yt[0:16, 0, :], MULT
                    )

            def mm2a_mtile(state, m):
                ch, yt, Q8, Q4 = state
                hp = hps.tile([128, CHUNK], f32, tag="hps")
                for j in range(4):  # (wa_kj|wa_kj) @ (qa_kj|qb_kj)
                    nc.tensor.matmul(
                        hp[0:MT, :], w2a_sb[:, m, j, :, :],
                        Q8[:, j, :, :],
                        start=(j == 0), stop=False, perf_mode=DR,
                    )
                # qa@wb with k-tiles paired: (wb_k0|wb_k1)@(qa_k0|qa_k1)
                nc.tensor.matmul(
                    hp[0:MT, :], w2a_sb[:, m, 4, :, :],
                    Q8[:, 0:2, 0, :],
                    start=False, stop=False, perf_mode=DR,
                )
                nc.tensor.matmul(
                    hp[0:MT, :], w2a_sb[:, m, 5, :, :],
                    Q8[:, 2:4, 0, :],
                    start=False, stop=False, perf_mode=DR,
                )
                # k-tile 4 (16 quad + 32 y + ones) in bf16, exact
                nc.tensor.matmul(
                    hp[0:MT, :], w2a4_sb[0:49, m, :], Q4[0:49, :],
                    start=False, stop=True,
                )
                h2 = h2p.tile([MT, CHUNK], bf16, tag="h2")
                nc.scalar.activation(
                    h2[:, :], hp[0:MT, :], Tanh, scale=1.0 / SCALE_W
                )
                return h2

            def mm2b(ch, h2list):
                # flipped: stationary = h2 b-slices (bf16), moving = W2b
                # single PSUM tile holds all 4 bt accumulators; zero it
                # explicitly and accumulate-only, since a start=True on one
                # bt slice zeroes the whole bank region (wiping the others)
                ob = opsp.tile([128, 4, D], f32, tag="ob")
                # start=True zeroes the whole bank region (verified identical
                # on HW and interp), so only the very first matmul starts and
                # the rest accumulate -- no explicit memset needed
                for t in range(NMT):
                    for bt in range(4):
                        nc.tensor.matmul(
                            ob[:, bt, :],
                            h2list[t][:, bt * 128:(bt + 1) * 128],
                            w2b_sb[0:MT, t, :],
                            start=(t == 0 and bt == 0),
                            stop=(t == NMT - 1 and bt == 3),
                            skip_group_check=True,
                        )
                osb = osp.tile([128, 4, D], f32, tag="osb")
                if opts.get("outcopy_dve", True):
                    nc.vector.tensor_copy(osb[:, :, :], ob[:, :, :])
                else:
                    nc.scalar.copy(osb[:, :, :], ob[:, :, :])
                nc.sync.dma_start(OUT[:, ch, :, :], osb[:, :, :])

            # Software pipeline, per iteration i:
            #   load(i); quad-build for chunk i interleaved with mm2a+tanh
            #   for chunk i-2; mm2b+store for chunk i-3. Interleaving keeps
            #   PE continuously busy (pstate ramp) while the gather->mult->
            #   split chain for chunk i drains on DVE/Pool.
            steps = opts.get("steps") or [
                ("a", 0), ("q", 0), ("a", 1), ("q", 1), ("a", 2),
                ("q", 2), ("a", 3), ("q", 3), ("a", 4), ("q", 4),
                ("a", 5)]
            states = {}
            h2s = {}
            for i in range(NCH + 3):
                st_f = None
                if i < NCH:
                    st_f = load(i)
                    states[i] = st_f
                st_a = states.get(i - 2)
                h2list = []
                for kind, idx in steps:
                    if kind == "q" and st_f is not None:
                        quad_tile(st_f, idx)
                    elif kind == "a" and st_a is not None:
                        h2list.append(mm2a_mtile(st_a, idx))
                if st_a is not None:
                    h2s[i - 2] = h2list
                    del states[i - 2]
                if (i - 3) in h2s:
                    mm2b(i - 3, h2s.pop(i - 3))

    nc.compile()
    return nc


def _host_prep(inp):
    import ml_dtypes

    def q8(x):
        return np.asarray(x, np.float32).astype(ml_dtypes.float8_e4m3)

    y = np.asarray(inp["y"], dtype=np.float32)
    rows, cols = np.tril_indices(D)
    perm = np.arange(Q)
    perm[496:512], perm[512:528] = (
        np.arange(512, 528).copy(), np.arange(496, 512).copy(),
    )
    rows = rows[perm]
    cols = cols[perm]

    # gather selection blocks: [32, 9, 2, 128] (R|R) / (C|C) pairs
    RCm = np.zeros((32, 9, 2, 128), np.float32)
    for t in range(4):
        qs = np.arange(t * 128, (t + 1) * 128)
        RCm[rows[qs], 2 * t, :, np.arange(128)] = 1.0
        RCm[cols[qs], 2 * t + 1, :, np.arange(128)] = 1.0
    RCm[rows[512 + np.arange(16)], 8, :, np.arange(16)] = 1.0

    # W' = scaled first-layer weights on the 561-row k-space x 756 col-space
    W2a = np.asarray(inp["W2a"], np.float32)[perm]     # [528, 700]
    W1a = np.asarray(inp["W1a"], np.float32)           # [32, 50]
    b2a = np.asarray(inp["b2a"], np.float32)
    b1a = np.asarray(inp["b1a"], np.float32)
    Wp = np.zeros((561, NMT, MT), np.float32)
    Hfull = np.zeros((561, 750), np.float32)
    Hfull[0:512, 0:700] = SCALE_W * W2a[0:512]
    Hfull[512:528, 0:700] = SCALE_W * W2a[512:528]
    Hfull[528:560, 700:750] = SCALE_W * W1a
    Hfull[560, 0:700] = SCALE_W * b2a
    Hfull[560, 700:750] = SCALE_W * b1a
    for m in range(NMT):
        Wp[:, m, 0:125] = Hfull[:, m * 125:(m + 1) * 125]
    Wp[560, 0, 125] = ONES_COL_RAW  # drives tanh-ones row for output bias

    wa = q8(Wp[0:512]).astype(np.float32)
    wb = q8(Wp[0:512] - wa).astype(np.float32)
    # pack [128, NMT, 6, 2, MT]: k-tile row -> partition
    W2A8 = np.zeros((128, NMT, 6, 2, MT), np.float32)
    for j in range(4):
        W2A8[:, :, j, 0, :] = wa[j * 128:(j + 1) * 128]
        W2A8[:, :, j, 1, :] = wa[j * 128:(j + 1) * 128]
    W2A8[:, :, 4, 0, :] = wb[0:128]
    W2A8[:, :, 4, 1, :] = wb[128:256]
    W2A8[:, :, 5, 0, :] = wb[256:384]
    W2A8[:, :, 5, 1, :] = wb[384:512]
    # k-tile 4 (16 quad + 32 y + ones) goes in bf16, unsplit
    W2A4T = np.zeros((64, NMT, MT), np.float32)
    W2A4T[0:49] = Wp[512:561]

    # W2b': [126 rows, 6 k-tiles, 32], row 125 of tile0 = output bias
    W2b = np.asarray(inp["W2b"], np.float32)
    W1b = np.asarray(inp["W1b"], np.float32)
    bo = np.asarray(inp["b1b"], np.float32) + np.asarray(inp["b2b"], np.float32)
    Vfull = np.concatenate([W2b, W1b], axis=0)         # [750, 32]
    W2B16 = np.zeros((128, NMT, D), np.float32)
    for t in range(NMT):
        W2B16[0:125, t, :] = Vfull[t * 125:(t + 1) * 125]
    W2B16[125, 0, :] = bo

    shared = {
        "W2A8": q8(W2A8),
        "W2A4T": W2A4T.astype(ml_dtypes.bfloat16),
        "W2B16": W2B16.astype(ml_dtypes.bfloat16),
        "RC8": q8(RCm),
    }
    yTs = []
    for i in range(N_CORES):
        yT = np.ascontiguousarray(y[i * BC:(i + 1) * BC].T)  # [32, BC]
        ya = q8(yT)
        yb = q8(yT - ya.astype(np.float32))
        yt8 = np.zeros((33, 2, BC), ml_dtypes.float8_e4m3)
        yt8[0:32, 0, :] = ya
        yt8[0:32, 1, :] = yb
        yt8[32, 0, :] = 1.0
        yt16 = np.ones((33, BC), ml_dtypes.bfloat16)
        yt16[0:32, :] = yT.astype(ml_dtypes.bfloat16)
        yTs.append((yt8, yt16))
    return shared, yTs


def kernel(**inputs):
    from concourse.bass_utils import run_bass_kernel_spmd

    if "nc" not in _CACHE:
        _CACHE["nc"] = _build_nc()
    nc = _CACHE["nc"]

    shared, yTs = _host_prep(inputs)
    in_maps = [
        dict(shared, yT8=yTs[i][0], yT16=yTs[i][1]) for i in range(N_CORES)
    ]
    try:
        res = run_bass_kernel_spmd(nc, in_maps, core_ids=list(range(N_CORES)))
    except ModuleNotFoundError:
        import os
        os.environ["BASS_NEVER_TRACE"] = "1"
        res = run_bass_kernel_spmd(nc, in_maps, core_ids=list(range(N_CORES)))
    _CACHE["last_result"] = res

    outs = []
    for r in res.results:
        arr = np.asarray(r["out"])  # [128, NCH, 4, D]
        outs.append(
            np.ascontiguousarray(
                arr.transpose(1, 2, 0, 3).reshape(BC, D)
            )
        )
    return np.ascontiguousarray(np.concatenate(outs, axis=0).astype(np.float32))
